# revision 38
# baseline (speedup 1.0000x reference)
"""Multi-head attention (B=2, N=2048, D=1024, H=16, HD=64) on 8 trn2 NeuronCores.

Sharding: data-parallel over batch (2) x tensor-parallel over head groups (4).
Core c handles batch b=c//4, heads 4*(c%4)..4*(c%4)+3. Each core computes
Q/K/V projections for its head slice, attention, and a partial output
projection (its heads' rows of Wo); the host sums the 4 partials per batch
and adds bo.

Device layout strategy: everything lives feature-on-partitions ("transposed")
so no on-device transposes are needed:
  - host passes X[b].T; Q^T/K^T computed as (W^T X^T) with W as stationary.
  - V computed in native [token, d] layout (X^T tiles as stationary).
  - scores computed as S^T[j, i] (key j on partitions) so the mask bias is a
    per-partition scalar and softmax normalization is deferred:
    E^T = exp(S/8 + maskbias) via one ScalarE activation (PSUM->SBUF).
  - ctx^T[d, i] = sum_j V_aug[j, d] E^T[j, i]; V_aug has a ones column so the
    softmax denominator rides along as ctx row 64.
  - normalization multiplies ctx^T by 1/denom broadcast via a tiny PE matmul.
  - out^T = Wo^T ctx^T accumulated over head pairs; host transposes back.
"""

import sys

if "/opt/trn_rl_repo" not in sys.path:
    sys.path.insert(0, "/opt/trn_rl_repo")

import ml_dtypes
import numpy as np

import concourse.bacc as bacc
import concourse.mybir as mybir
import concourse.tile as tile

B, N, D = 2, 2048, 1024
H, HD = 16, 64
HG = 4  # head groups (tensor parallel)
HPG = H // HG  # heads per group = 4
DG = HPG * HD  # feature slice per group = 256

F32 = mybir.dt.float32
# Matmul datapath dtype: bf16 runs 1 cycle/row on the PE and halves DMA
# traffic + SBUF footprint vs fp32r. PSUM accumulation stays fp32.
MMT = mybir.dt.bfloat16
# fp32r kept for the small norm-broadcast matmuls (r values stay fp32).
F32R = mybir.dt.float32r


def _mm_ap(ap):
    return ap


def build_program(loop_iters: int = 1):
    nc = bacc.Bacc("TRN2", target_bir_lowering=False)

    xt = nc.dram_tensor("xt", [D, N], MMT, kind="ExternalInput")
    # weights mt-major so each head-pair half loads as one contiguous DMA
    wq = nc.dram_tensor("wq", [128, 2, 8, 128], MMT, kind="ExternalInput")
    wk = nc.dram_tensor("wk", [128, 2, 8, 128], MMT, kind="ExternalInput")
    wv = nc.dram_tensor("wv", [128, 8, DG], MMT, kind="ExternalInput")
    wo = nc.dram_tensor("wo", [128, 2, D], MMT, kind="ExternalInput")
    # merged constants, one DMA: [:,0:2]=bq, [:,2:4]=bk, [:,4:20]=mask bias,
    # [:,20:148]=ones (f32r for PE broadcasts), [0,148:404]=bv row
    cst = nc.dram_tensor("cst", [128, 404], F32R, kind="ExternalInput")
    onesin = nc.dram_tensor("onesin", [128, 64], MMT, kind="ExternalInput")
    outp = nc.dram_tensor("outp", [D, N], MMT, kind="ExternalOutput")

    with tile.TileContext(nc) as tc, nc.allow_low_precision(
        reason="fp32r matmul datapath; accumulation stays fp32 in PSUM"
    ):
        import contextlib

        ctx = contextlib.ExitStack()
        with ctx:
            const = ctx.enter_context(tc.tile_pool(name="const", bufs=1))
            big = ctx.enter_context(tc.tile_pool(name="big", bufs=5))
            xtcp = ctx.enter_context(tc.tile_pool(name="xtcp", bufs=4))
            qk = ctx.enter_context(tc.tile_pool(name="qk", bufs=1))
            epool = ctx.enter_context(tc.tile_pool(name="epool", bufs=6))
            rpool = ctx.enter_context(tc.tile_pool(name="rpool", bufs=2))
            psum_b = ctx.enter_context(
                tc.tile_pool(name="psum_b", bufs=2, space="PSUM")
            )
            psum_c = ctx.enter_context(
                tc.tile_pool(name="psum_c", bufs=2, space="PSUM")
            )

            loop_cm = (
                tc.For_i(0, loop_iters, 1)
                if loop_iters > 1
                else contextlib.nullcontext()
            )
            with loop_cm:
                # ---- phase 1 loads, all on the sync queue in first-use
                # order. Each DMA instruction costs ~625ns of serialized
                # HWDGE descriptor generation, so batches are as coarse as
                # first-use order allows.
                wq_sb = big.tile([128, 2, 8, 128], MMT, tag="big")
                wk_sb = big.tile([128, 2, 8, 128], MMT, tag="big")
                wv_sb = big.tile([128, 8, DG], MMT, tag="big")
                xtc_t = [
                    xtcp.tile([128, 8, 512], MMT, tag="xtc", name="xtc")
                    for _ in range(4)
                ]
                xtc = [[xtc_t[c][:, kt, :] for kt in range(8)] for c in range(4)]

                def load_x(c, h0, h1):
                    nc.sync.dma_start(
                        out=xtc_t[c][:, h0:h1, :],
                        in_=xt[
                            h0 * 128 : h1 * 128, c * 512 : (c + 1) * 512
                        ].rearrange("(kt p) col -> p kt col", p=128),
                    )

                nc.sync.dma_start(out=wq_sb[:, 0], in_=wq[:, 0])
                for h in range(0, 8, 2):
                    load_x(0, h, h + 2)

                cst_sb = const.tile([128, 404], F32R, tag="cst")
                nc.sync.dma_start(out=cst_sb, in_=cst[:, :])
                bq_sb = cst_sb[:, 0:2]
                bk_sb = cst_sb[:, 2:4]
                mb_sb = cst_sb[:, 4:20]
                ones = cst_sb[:, 20:148]
                bvr_sb = cst_sb[0:1, 148:404]
                wo_sb = const.tile([128, 2, D], MMT, tag="wo")
                bv_bc = const.tile([128, DG], F32, tag="bvbc")

                def emit_bv_bcast():
                    # bv broadcast to all 128 partitions via PE ones-matmul;
                    # deferred into the chain stream so it doesn't head-block
                    # the PE on the cst DMA
                    bv_ps = psum_b.tile([128, DG], F32, tag="bank")
                    nc.tensor.matmul(
                        bv_ps, _mm_ap(ones[0:1, 0:128]), _mm_ap(bvr_sb),
                        start=True, stop=True,
                    )
                    nc.vector.tensor_copy(bv_bc, bv_ps)

                nc.sync.dma_start(out=wk_sb[:, 0], in_=wk[:, 0])
                load_x(1, 0, 4)
                load_x(1, 4, 8)
                nc.sync.dma_start(out=wv_sb, in_=wv[:, :, :])

                qt_sb = [qk.tile([128, N], MMT, tag=f"qt{m}", name=f"qt{m}") for m in range(2)]
                kt_sb = [qk.tile([128, N], MMT, tag=f"kt{m}", name=f"kt{m}") for m in range(2)]
                # V with ones column appended per head: [128, jt, head, 65]
                v_sb = qk.tile([128, 16, HPG, HD + 1], MMT, tag="v")
                nc.scalar.dma_start(
                    out=v_sb[:, :, :, HD : HD + 1], in_=onesin[:, :]
                )
                load_x(2, 0, 8)
                # second head-pair weight columns: first consumer is
                # qk_chain(0/1, 1, *) around unit 11 (~30us in)
                nc.sync.dma_start(out=wq_sb[:, 1], in_=wq[:, 1])
                nc.sync.dma_start(out=wk_sb[:, 1], in_=wk[:, 1])
                load_x(3, 0, 8)
                nc.sync.dma_start(out=wo_sb, in_=wo[:, :, :])

                def qk_chain(proj, mt, nt):
                    w_sb, bias_sb, dst = (
                        (wq_sb, bq_sb, qt_sb) if proj == 0 else (wk_sb, bk_sb, kt_sb)
                    )
                    ps = psum_b.tile([128, 512], F32, tag="bank", name="qkps")
                    for kt in range(8):
                        nc.tensor.matmul(
                            ps,
                            _mm_ap(w_sb[:, mt, kt, :]),
                            _mm_ap(xtc[nt][kt]),
                            start=(kt == 0),
                            stop=(kt == 7),
                        )
                    nc.vector.tensor_scalar_add(
                        dst[mt][:, nt * 512 : (nt + 1) * 512],
                        ps,
                        bias_sb[:, mt : mt + 1],
                    )

                def v_chain(mt):
                    ps = psum_b.tile([128, DG], F32, tag="bank", name="vps")
                    for kt in range(8):
                        nc.tensor.matmul(
                            ps,
                            _mm_ap(
                                xtc[mt // 4][kt][
                                    :, (mt % 4) * 128 : (mt % 4 + 1) * 128
                                ]
                            ),
                            _mm_ap(wv_sb[:, kt, :]),
                            start=(kt == 0),
                            stop=(kt == 7),
                        )
                    nc.vector.tensor_tensor(
                        out=v_sb[:, mt, :, 0:HD],
                        in0=ps.rearrange("p (h d) -> p h d", h=HPG),
                        in1=bv_bc.rearrange("p (h d) -> p h d", h=HPG),
                        op=mybir.AluOpType.add,
                    )

                # chains needed before the unit stream starts (unit 0 only
                # needs Q nt=0/1, K nt=0; V(0) and K nt=1 follow as inserts
                # so the exp stream starts as early as possible)
                for fn in (
                    lambda: qk_chain(0, 0, 0),
                    lambda: qk_chain(1, 0, 0),
                    lambda: qk_chain(0, 0, 1),
                ):
                    fn()

                # remaining chains, spread through the unit stream (key =
                # iteration index at whose END the chain is emitted; each must
                # precede its first consumer unit)
                inserts = {}
                inserts.setdefault(0, []).append(emit_bv_bcast)
                inserts.setdefault(0, []).append(lambda: v_chain(0))
                for i in range(1, 16):
                    inserts.setdefault(i, []).append(lambda m=i: v_chain(m))
                inserts.setdefault(2, []).append(lambda: qk_chain(1, 0, 1))
                inserts.setdefault(6, []).append(lambda: qk_chain(1, 0, 2))
                inserts.setdefault(10, []).append(lambda: qk_chain(1, 0, 3))
                inserts.setdefault(11, []).append(lambda: qk_chain(0, 1, 0))
                inserts.setdefault(12, []).append(lambda: qk_chain(1, 1, 0))
                inserts.setdefault(13, []).append(lambda: qk_chain(0, 1, 1))
                # block boundaries (16/17, 32/33, 48/49) are where the ctx
                # pipeline refills — park chain work there so PE stays fed
                inserts.setdefault(16, []).append(lambda: qk_chain(1, 1, 1))
                inserts.setdefault(17, []).append(lambda: qk_chain(1, 1, 2))
                inserts.setdefault(25, []).append(lambda: qk_chain(1, 1, 3))
                inserts.setdefault(28, []).append(lambda: qk_chain(0, 0, 2))
                inserts.setdefault(30, []).append(lambda: qk_chain(0, 0, 3))
                inserts.setdefault(32, []).append(lambda: qk_chain(0, 1, 2))
                inserts.setdefault(33, []).append(lambda: qk_chain(0, 1, 3))


                # ---- phase 2: attention, software-pipelined emission ----
                ctxn = [
                    qk.tile([128, N], MMT, tag=f"ctxn{m}", name=f"ctxn{m}")
                    for m in range(2)
                ]

                blocks = [(ih, hp) for ih in range(2) for hp in range(2)]
                units = [
                    (b_idx, ih, hp, jt)
                    for b_idx, (ih, hp) in enumerate(blocks)
                    for jt in range(16)
                ]
                ctx_ps_of = {}
                unit_e = {}

                def emit_s_exp(u):
                    b_idx, ih, hp, jt = u
                    e_sb = [
                        epool.tile([128, 1024], MMT, tag="e", name="esb")
                        for _ in range(2)
                    ]
                    s_ps2 = [
                        psum_b.tile([128, 1024], F32, tag="bank", name="sps")
                        for _ in range(2)
                    ]
                    for h2 in range(2):
                        for it in range(2):
                            nc.tensor.matmul(
                                s_ps2[h2][:, it * 512 : (it + 1) * 512],
                                _mm_ap(
                                    kt_sb[hp][
                                        h2 * 64 : (h2 + 1) * 64,
                                        jt * 128 : (jt + 1) * 128,
                                    ]
                                ),
                                _mm_ap(
                                    qt_sb[hp][
                                        h2 * 64 : (h2 + 1) * 64,
                                        ih * 1024 + it * 512 : ih * 1024
                                        + (it + 1) * 512,
                                    ]
                                ),
                                start=True,
                                stop=True,
                            )
                    for h2 in range(2):
                        nc.scalar.activation(
                            out=e_sb[h2],
                            in_=s_ps2[h2],
                            func=mybir.ActivationFunctionType.Exp,
                            bias=mb_sb[:, jt : jt + 1],
                            scale=0.125,
                        )
                    unit_e[u] = e_sb

                def emit_ctx(u):
                    b_idx, ih, hp, jt = u
                    if b_idx not in ctx_ps_of:
                        ctx_ps_of[b_idx] = [
                            psum_c.tile([HD + 1, 1024], F32, tag="ctx", name="ctxps")
                            for _ in range(2)
                        ]
                    ctx_ps = ctx_ps_of[b_idx]
                    e_sb = unit_e.pop(u)
                    for h2 in range(2):
                        for it in range(2):
                            nc.tensor.matmul(
                                ctx_ps[h2][:, it * 512 : (it + 1) * 512],
                                _mm_ap(v_sb[:, jt, 2 * hp + h2, :]),
                                _mm_ap(e_sb[h2][:, it * 512 : (it + 1) * 512]),
                                start=(jt == 0),
                                stop=(jt == 15),
                                skip_group_check=True,
                            )

                def bcast_r(r_sb, lo, width):
                    # broadcast r_sb[64, lo:lo+width] to partitions 0-63 via
                    # a PE ones-matmul; psum->sbuf copies alternate Act/DVE
                    # so the block-boundary latency chain stays short
                    for it in range((width + 511) // 512):
                        c0 = lo + it * 512
                        c1 = min(lo + width, c0 + 512)
                        rp = psum_b.tile([64, 512], F32, tag="bank", name="rp")
                        nc.tensor.matmul(
                            rp[:, 0 : c1 - c0],
                            _mm_ap(ones[64:65, 0:64]),
                            _mm_ap(r_sb[64:65, c0:c1]),
                            start=True,
                            stop=True,
                            tile_position=(64, 0),
                        )
                        ce = nc.scalar.copy if it % 2 == 0 else nc.vector.tensor_copy
                        ce(r_sb[0:64, c0:c1], rp[:, 0 : c1 - c0])

                def emit_norm(b_idx):
                    ih, hp = blocks[b_idx]
                    ctx_ps = ctx_ps_of[b_idx]
                    for h2 in (1, 0):
                        r_sb = rpool.tile([65, 1024], F32R, tag="r", name="rsb")
                        nc.vector.reciprocal(
                            out=r_sb[64:65, :], in_=ctx_ps[h2][64:65, :]
                        )
                        bcast_r(r_sb, 0, 1024)
                        if h2 == 0:
                            nc.vector.tensor_tensor(
                                out=ctxn[hp][0:64, ih * 1024 : (ih + 1) * 1024],
                                in0=ctx_ps[0][0:64, :],
                                in1=r_sb[0:64, :],
                                op=mybir.AluOpType.mult,
                            )
                        else:
                            tmp = big.tile([64, 1024], MMT, tag="big", name="tmp")
                            nc.vector.tensor_tensor(
                                out=tmp,
                                in0=ctx_ps[1][0:64, :],
                                in1=r_sb[0:64, :],
                                op=mybir.AluOpType.mult,
                            )
                            # partition shift 0-63 -> 64-127 via SBUF->SBUF DMA
                            nc.sync.dma_start(
                                out=ctxn[hp][64:128, ih * 1024 : (ih + 1) * 1024],
                                in_=tmp,
                            )

                def emit_outproj(ih, mo_list=None, copy_eng=None):
                    for mo in (range(8) if mo_list is None else mo_list):
                        ps = psum_b.tile([128, 1024], F32, tag="bank", name="ops")
                        for nt2 in range(2):
                            nt = 2 * ih + nt2
                            for kt in range(2):
                                nc.tensor.matmul(
                                    ps[:, nt2 * 512 : (nt2 + 1) * 512],
                                    _mm_ap(wo_sb[:, kt, mo * 128 : (mo + 1) * 128]),
                                    _mm_ap(ctxn[kt][:, nt * 512 : (nt + 1) * 512]),
                                    start=(kt == 0),
                                    stop=(kt == 1),
                                )
                        ob = big.tile([128, 1024], MMT, tag="big", name="ob")
                        (copy_eng or nc.vector.tensor_copy)(ob, ps)
                        nc.sync.dma_start(
                            out=outp[
                                mo * 128 : (mo + 1) * 128,
                                ih * 1024 : (ih + 1) * 1024,
                            ],
                            in_=ob,
                        )

                # ---- tail: final block (b3, ih=1/hp=1) normalized and
                # projected in 512-query chunks so the output projection
                # starts as soon as the first chunk's normalize lands.
                def emit_tail_norm(it):
                    ctx_ps = ctx_ps_of[3]
                    cols = slice(it * 512, (it + 1) * 512)
                    dst = slice(1024 + it * 512, 1024 + (it + 1) * 512)
                    for h2 in (1, 0):
                        r_sb = rpool.tile([65, 512], F32R, tag="r", name="rsb")
                        nc.vector.reciprocal(
                            out=r_sb[64:65, :], in_=ctx_ps[h2][64:65, cols]
                        )
                        bcast_r(r_sb, 0, 512)
                        if h2 == 0:
                            nc.vector.tensor_tensor(
                                out=ctxn[1][0:64, dst],
                                in0=ctx_ps[0][0:64, cols],
                                in1=r_sb[0:64, :],
                                op=mybir.AluOpType.mult,
                            )
                        else:
                            tmp = big.tile(
                                [64, 512], MMT, tag="big", name="tmp"
                            )
                            nc.vector.tensor_tensor(
                                out=tmp,
                                in0=ctx_ps[1][0:64, cols],
                                in1=r_sb[0:64, :],
                                op=mybir.AluOpType.mult,
                            )
                            nc.sync.dma_start(
                                out=ctxn[1][64:128, dst], in_=tmp
                            )

                def emit_tail_outproj(it, mo, copy_eng):
                    nt = 2 + it
                    ps = psum_b.tile([128, 512], F32, tag="bank", name="ops")
                    for kt in range(2):
                        nc.tensor.matmul(
                            ps,
                            _mm_ap(wo_sb[:, kt, mo * 128 : (mo + 1) * 128]),
                            _mm_ap(ctxn[kt][:, nt * 512 : (nt + 1) * 512]),
                            start=(kt == 0),
                            stop=(kt == 1),
                        )
                    ob = big.tile([128, 512], MMT, tag="big", name="ob")
                    copy_eng(ob, ps)
                    nc.sync.dma_start(
                        out=outp[
                            mo * 128 : (mo + 1) * 128,
                            nt * 512 : (nt + 1) * 512,
                        ],
                        in_=ob,
                    )

                extras = {}
                for b_idx, (ih, hp) in enumerate(blocks[:3]):
                    last = 16 * (b_idx + 1) - 1
                    extras.setdefault(last + 1, []).append(
                        lambda b=b_idx: emit_norm(b)
                    )
                # ih=0 output projection: spread over the b2 stream and the
                # late (insert-starved) b3 stream
                for j, mo in enumerate(range(8)):
                    pos = 35 + 3 * j if j < 4 else 49 + 3 * (j - 4)
                    extras.setdefault(pos, []).append(
                        lambda m=mo: emit_outproj(0, [m])
                    )

                next_ctx = 0
                for i, u in enumerate(units):
                    emit_s_exp(u)
                    # ctx for a block's first unit lags one extra iteration
                    # so the PE doesn't stall on the previous block's PSUM
                    # (freed only once its normalize has read it)
                    while next_ctx < i and (
                        next_ctx <= i - 2 or units[next_ctx][3] != 0
                    ):
                        emit_ctx(units[next_ctx])
                        next_ctx += 1
                    for fn in inserts.get(i, []):
                        fn()
                    for fn in extras.get(i, []):
                        fn()
                while next_ctx < len(units):
                    emit_ctx(units[next_ctx])
                    next_ctx += 1
                emit_tail_norm(0)
                emit_tail_norm(1)
                for it in range(2):
                    for mo in range(8):
                        ce = nc.scalar.copy if mo % 2 == 0 else nc.vector.tensor_copy
                        emit_tail_outproj(it, mo, ce)

    nc.finalize()
    return nc


_NC_CACHE = None


def _get_program():
    global _NC_CACHE
    if _NC_CACHE is None:
        _NC_CACHE = build_program()
    return _NC_CACHE


def make_in_maps(X, mask, Wq, bq, Wk, bk, Wv, bv, Wo, bo):
    BF16 = ml_dtypes.bfloat16
    X = np.asarray(X, dtype=np.float32)
    mask = np.asarray(mask, dtype=np.float32)
    in_maps = []
    xts = [np.ascontiguousarray(X[b].T).astype(BF16) for b in range(B)]
    mbs = [
        np.ascontiguousarray((-1e6 * (1.0 - mask[b])).reshape(16, 128).T)
        for b in range(B)
    ]
    for c in range(8):
        b, g = c // HG, c % HG
        sl = slice(g * DG, (g + 1) * DG)
        # [p, mt, kt, j] = W[kt*128+p, mt*128+j]
        wq_s = np.ascontiguousarray(
            np.asarray(Wq[:, sl]).reshape(8, 128, 2, 128).transpose(1, 2, 0, 3)
        )
        wk_s = np.ascontiguousarray(
            np.asarray(Wk[:, sl]).reshape(8, 128, 2, 128).transpose(1, 2, 0, 3)
        )
        wv_s = np.ascontiguousarray(
            np.asarray(Wv[:, sl]).reshape(8, 128, DG).transpose(1, 0, 2)
        )
        # Wo rows for this group, pair-packed: [64*h2+p, kt, o] = Wo[g*256+(2kt+h2)*64+p, o]
        wo_s = np.ascontiguousarray(
            np.asarray(Wo[sl, :]).reshape(2, 2, 64, D).transpose(1, 2, 0, 3)
            .reshape(128, 2, D)
        )
        cst_a = np.zeros((128, 404), dtype=np.float32)
        cst_a[:, 0:2] = np.asarray(bq[sl]).reshape(2, 128).T
        cst_a[:, 2:4] = np.asarray(bk[sl]).reshape(2, 128).T
        cst_a[:, 4:20] = mbs[b]
        cst_a[:, 20:148] = 1.0
        cst_a[0, 148:404] = np.asarray(bv[sl])
        in_maps.append(
            {
                "xt": xts[b],
                "onesin": np.ones((128, 64), dtype=BF16),
                "cst": cst_a,
                "wq": wq_s.astype(BF16),
                "wk": wk_s.astype(BF16),
                "wv": wv_s.astype(BF16),
                "wo": wo_s.astype(BF16),
            }
        )
    return in_maps


def gather_output(results, bo):
    out = np.zeros((B, N, D), dtype=np.float32)
    for c in range(8):
        out[c // HG] += results[c]["outp"].astype(np.float32).T
    out += np.asarray(bo, dtype=np.float32)
    return out


def kernel(**inputs):
    from concourse import bass_utils

    nc = _get_program()
    in_maps = make_in_maps(**inputs)
    res = bass_utils.run_bass_kernel_spmd(nc, in_maps, core_ids=list(range(8)))
    return gather_output(res.results, inputs["bo"])



# revision 46
# speedup vs baseline: 10.2774x; 10.2774x over previous
"""Multi-head attention (B=2, N=2048, D=1024, H=16, HD=64) on 8 trn2 NeuronCores.

Sharding: data-parallel over batch (2) x tensor-parallel over head groups (4).
Core c handles batch b=c//4, heads 4*(c%4)..4*(c%4)+3. Each core computes
Q/K/V projections for its head slice, attention, and a partial output
projection (its heads' rows of Wo); the host sums the 4 partials per batch
and adds bo.

Device layout strategy: everything lives feature-on-partitions ("transposed")
so no on-device transposes are needed:
  - host passes X[b].T; Q^T/K^T computed as (W^T X^T) with W as stationary.
  - V computed in native [token, d] layout (X^T tiles as stationary).
  - scores computed as S^T[j, i] (key j on partitions) so the mask bias is a
    per-partition scalar and softmax normalization is deferred:
    E^T = exp(S/8 + maskbias) via one ScalarE activation (PSUM->SBUF).
  - ctx^T[d, i] = sum_j V_aug[j, d] E^T[j, i]; V_aug has a ones column so the
    softmax denominator rides along as ctx row 64.
  - normalization multiplies ctx^T by 1/denom broadcast via a tiny PE matmul.
  - out^T = Wo^T ctx^T accumulated over head pairs; host transposes back.
"""

import sys

if "/opt/trn_rl_repo" not in sys.path:
    sys.path.insert(0, "/opt/trn_rl_repo")

import ml_dtypes
import numpy as np

import concourse.bacc as bacc
import concourse.mybir as mybir
import concourse.tile as tile

B, N, D = 2, 2048, 1024
H, HD = 16, 64
HG = 4  # head groups (tensor parallel)
HPG = H // HG  # heads per group = 4
DG = HPG * HD  # feature slice per group = 256

F32 = mybir.dt.float32
# Matmul datapath dtype: bf16 runs 1 cycle/row on the PE and halves DMA
# traffic + SBUF footprint vs fp32r. PSUM accumulation stays fp32.
MMT = mybir.dt.bfloat16
# fp32r kept for the small norm-broadcast matmuls (r values stay fp32).
F32R = mybir.dt.float32r


def _mm_ap(ap):
    return ap


def build_program(loop_iters: int = 1):
    nc = bacc.Bacc("TRN2", target_bir_lowering=False)

    xt = nc.dram_tensor("xt", [D, N], MMT, kind="ExternalInput")
    # weights mt-major so each head-pair half loads as one contiguous DMA
    wq = nc.dram_tensor("wq", [128, 2, 8, 128], MMT, kind="ExternalInput")
    wk = nc.dram_tensor("wk", [128, 2, 8, 128], MMT, kind="ExternalInput")
    wv = nc.dram_tensor("wv", [128, 8, DG], MMT, kind="ExternalInput")
    wo = nc.dram_tensor("wo", [128, 2, D], MMT, kind="ExternalInput")
    # merged constants, one DMA: [:,0:2]=bq, [:,2:4]=bk, [:,4:20]=mask bias
    cst = nc.dram_tensor("cst", [128, 20], F32, kind="ExternalInput")
    # bf16 ones: v ones-column source + stationary for PE broadcasts
    onesin = nc.dram_tensor("onesin", [128, 128], MMT, kind="ExternalInput")
    bvr = nc.dram_tensor("bvr", [1, DG], MMT, kind="ExternalInput")
    outp = nc.dram_tensor("outp", [D, N], MMT, kind="ExternalOutput")

    with tile.TileContext(nc) as tc, nc.allow_low_precision(
        reason="fp32r matmul datapath; accumulation stays fp32 in PSUM"
    ):
        import contextlib

        ctx = contextlib.ExitStack()
        with ctx:
            const = ctx.enter_context(tc.tile_pool(name="const", bufs=1))
            big = ctx.enter_context(tc.tile_pool(name="big", bufs=5))
            xtcp = ctx.enter_context(tc.tile_pool(name="xtcp", bufs=4))
            qk = ctx.enter_context(tc.tile_pool(name="qk", bufs=1))
            epool = ctx.enter_context(tc.tile_pool(name="epool", bufs=6))
            rpool = ctx.enter_context(tc.tile_pool(name="rpool", bufs=2))
            psum_b = ctx.enter_context(
                tc.tile_pool(name="psum_b", bufs=2, space="PSUM")
            )
            psum_c = ctx.enter_context(
                tc.tile_pool(name="psum_c", bufs=2, space="PSUM")
            )

            loop_cm = (
                tc.For_i(0, loop_iters, 1)
                if loop_iters > 1
                else contextlib.nullcontext()
            )
            with loop_cm:
                # ---- phase 1 loads, all on the sync queue in first-use
                # order. Each DMA instruction costs ~625ns of serialized
                # HWDGE descriptor generation, so batches are as coarse as
                # first-use order allows.
                wq_sb = big.tile([128, 2, 8, 128], MMT, tag="big")
                wk_sb = big.tile([128, 2, 8, 128], MMT, tag="big")
                wv_sb = big.tile([128, 8, DG], MMT, tag="big")
                xtc_t = [
                    xtcp.tile([128, 8, 512], MMT, tag="xtc", name="xtc")
                    for _ in range(4)
                ]
                xtc = [[xtc_t[c][:, kt, :] for kt in range(8)] for c in range(4)]

                def load_x(c, h0, h1):
                    nc.sync.dma_start(
                        out=xtc_t[c][:, h0:h1, :],
                        in_=xt[
                            h0 * 128 : h1 * 128, c * 512 : (c + 1) * 512
                        ].rearrange("(kt p) col -> p kt col", p=128),
                    )

                # critical prefix: exactly what unit 0's score matmuls need
                nc.sync.dma_start(out=wq_sb[:, 0], in_=wq[:, 0])
                load_x(0, 0, 4)
                load_x(0, 4, 8)
                nc.sync.dma_start(out=wk_sb[:, 0], in_=wk[:, 0])
                load_x(1, 0, 8)

                cst_sb = const.tile([128, 20], F32, tag="cst")
                nc.sync.dma_start(out=cst_sb, in_=cst[:, :])
                ones = const.tile([128, 128], MMT, tag="ones")
                nc.sync.dma_start(out=ones, in_=onesin[:, :])
                bvr_sb = const.tile([1, DG], MMT, tag="bvr")
                nc.sync.dma_start(out=bvr_sb, in_=bvr[:, :])
                bq_sb = cst_sb[:, 0:2]
                bk_sb = cst_sb[:, 2:4]
                mb_sb = cst_sb[:, 4:20]
                wo_sb = const.tile([128, 2, D], MMT, tag="wo")
                bv_bc = const.tile([128, DG], F32, tag="bvbc")

                def emit_bv_bcast():
                    # bv broadcast to all 128 partitions via PE ones-matmul;
                    # deferred into the chain stream so it doesn't head-block
                    # the PE on the cst DMA
                    bv_ps = psum_b.tile([128, DG], F32, tag="bank")
                    nc.tensor.matmul(
                        bv_ps, _mm_ap(ones[0:1, 0:128]), _mm_ap(bvr_sb[0:1, :]),
                        start=True, stop=True,
                    )
                    nc.vector.tensor_copy(bv_bc, bv_ps)

                nc.sync.dma_start(out=wv_sb, in_=wv[:, :, :])

                qt_sb = [qk.tile([128, N], MMT, tag=f"qt{m}", name=f"qt{m}") for m in range(2)]
                kt_sb = [qk.tile([128, N], MMT, tag=f"kt{m}", name=f"kt{m}") for m in range(2)]
                # V with ones column appended per head: [128, jt, head, 65]
                v_sb = qk.tile([128, 16, HPG, HD + 1], MMT, tag="v")
                nc.sync.dma_start(
                    out=v_sb[:, :, :, HD : HD + 1], in_=onesin[:, 0:64]
                )
                load_x(2, 0, 8)
                # second head-pair weight columns: first consumer is
                # qk_chain(0/1, 1, *) around unit 11 (~30us in)
                nc.sync.dma_start(out=wq_sb[:, 1], in_=wq[:, 1])
                nc.sync.dma_start(out=wk_sb[:, 1], in_=wk[:, 1])
                load_x(3, 0, 8)
                nc.sync.dma_start(out=wo_sb, in_=wo[:, :, :])

                def qk_chain(proj, mt, nt):
                    w_sb, bias_sb, dst = (
                        (wq_sb, bq_sb, qt_sb) if proj == 0 else (wk_sb, bk_sb, kt_sb)
                    )
                    ps = psum_b.tile([128, 512], F32, tag="bank", name="qkps")
                    for kt in range(8):
                        nc.tensor.matmul(
                            ps,
                            _mm_ap(w_sb[:, mt, kt, :]),
                            _mm_ap(xtc[nt][kt]),
                            start=(kt == 0),
                            stop=(kt == 7),
                        )
                    nc.vector.tensor_scalar_add(
                        dst[mt][:, nt * 512 : (nt + 1) * 512],
                        ps,
                        bias_sb[:, mt : mt + 1],
                    )

                def v_chain(mt):
                    ps = psum_b.tile([128, DG], F32, tag="bank", name="vps")
                    for kt in range(8):
                        nc.tensor.matmul(
                            ps,
                            _mm_ap(
                                xtc[mt // 4][kt][
                                    :, (mt % 4) * 128 : (mt % 4 + 1) * 128
                                ]
                            ),
                            _mm_ap(wv_sb[:, kt, :]),
                            start=(kt == 0),
                            stop=(kt == 7),
                        )
                    nc.vector.tensor_tensor(
                        out=v_sb[:, mt, :, 0:HD],
                        in0=ps.rearrange("p (h d) -> p h d", h=HPG),
                        in1=bv_bc.rearrange("p (h d) -> p h d", h=HPG),
                        op=mybir.AluOpType.add,
                    )

                # chains needed before the unit stream starts (unit 0 only
                # needs Q nt=0/1, K nt=0; V(0) and K nt=1 follow as inserts
                # so the exp stream starts as early as possible)
                for fn in (
                    lambda: qk_chain(0, 0, 0),
                    lambda: qk_chain(1, 0, 0),
                    lambda: qk_chain(0, 0, 1),
                ):
                    fn()

                # remaining chains, spread through the unit stream (key =
                # iteration index at whose END the chain is emitted; each must
                # precede its first consumer unit)
                inserts = {}
                inserts.setdefault(0, []).append(emit_bv_bcast)
                inserts.setdefault(0, []).append(lambda: v_chain(0))
                for i in range(1, 16):
                    inserts.setdefault(i, []).append(lambda m=i: v_chain(m))
                inserts.setdefault(2, []).append(lambda: qk_chain(1, 0, 1))
                inserts.setdefault(6, []).append(lambda: qk_chain(1, 0, 2))
                inserts.setdefault(10, []).append(lambda: qk_chain(1, 0, 3))
                inserts.setdefault(11, []).append(lambda: qk_chain(0, 1, 0))
                inserts.setdefault(12, []).append(lambda: qk_chain(1, 1, 0))
                inserts.setdefault(13, []).append(lambda: qk_chain(0, 1, 1))
                # block boundaries (16/17, 32/33, 48/49) are where the ctx
                # pipeline refills — park chain work there so PE stays fed
                inserts.setdefault(16, []).append(lambda: qk_chain(1, 1, 1))
                inserts.setdefault(17, []).append(lambda: qk_chain(1, 1, 2))
                inserts.setdefault(25, []).append(lambda: qk_chain(1, 1, 3))
                inserts.setdefault(28, []).append(lambda: qk_chain(0, 0, 2))
                inserts.setdefault(30, []).append(lambda: qk_chain(0, 0, 3))
                inserts.setdefault(32, []).append(lambda: qk_chain(0, 1, 2))
                inserts.setdefault(33, []).append(lambda: qk_chain(0, 1, 3))


                # ---- phase 2: attention, software-pipelined emission ----
                ctxn = [
                    qk.tile([128, N], MMT, tag=f"ctxn{m}", name=f"ctxn{m}")
                    for m in range(2)
                ]

                blocks = [(ih, hp) for ih in range(2) for hp in range(2)]
                units = [
                    (b_idx, ih, hp, jt)
                    for b_idx, (ih, hp) in enumerate(blocks)
                    for jt in range(16)
                ]
                ctx_ps_of = {}
                unit_e = {}

                def emit_s_exp(u):
                    b_idx, ih, hp, jt = u
                    e_sb = [
                        epool.tile([128, 1024], MMT, tag="e", name="esb")
                        for _ in range(2)
                    ]
                    s_ps2 = [
                        psum_b.tile([128, 1024], F32, tag="bank", name="sps")
                        for _ in range(2)
                    ]
                    for h2 in range(2):
                        for it in range(2):
                            nc.tensor.matmul(
                                s_ps2[h2][:, it * 512 : (it + 1) * 512],
                                _mm_ap(
                                    kt_sb[hp][
                                        h2 * 64 : (h2 + 1) * 64,
                                        jt * 128 : (jt + 1) * 128,
                                    ]
                                ),
                                _mm_ap(
                                    qt_sb[hp][
                                        h2 * 64 : (h2 + 1) * 64,
                                        ih * 1024 + it * 512 : ih * 1024
                                        + (it + 1) * 512,
                                    ]
                                ),
                                start=True,
                                stop=True,
                            )
                    for h2 in range(2):
                        nc.scalar.activation(
                            out=e_sb[h2],
                            in_=s_ps2[h2],
                            func=mybir.ActivationFunctionType.Exp,
                            bias=mb_sb[:, jt : jt + 1],
                            scale=0.125,
                        )
                    unit_e[u] = e_sb

                def emit_ctx(u):
                    b_idx, ih, hp, jt = u
                    if b_idx not in ctx_ps_of:
                        ctx_ps_of[b_idx] = [
                            psum_c.tile([HD + 1, 1024], F32, tag="ctx", name="ctxps")
                            for _ in range(2)
                        ]
                    ctx_ps = ctx_ps_of[b_idx]
                    e_sb = unit_e.pop(u)
                    for h2 in range(2):
                        for it in range(2):
                            nc.tensor.matmul(
                                ctx_ps[h2][:, it * 512 : (it + 1) * 512],
                                _mm_ap(v_sb[:, jt, 2 * hp + h2, :]),
                                _mm_ap(e_sb[h2][:, it * 512 : (it + 1) * 512]),
                                start=(jt == 0),
                                stop=(jt == 15),
                                skip_group_check=True,
                            )

                def bcast_r(r_sb, lo, width):
                    # broadcast r_sb[64, lo:lo+width] to partitions 0-63 via
                    # a PE ones-matmul; psum->sbuf copies alternate Act/DVE
                    # so the block-boundary latency chain stays short
                    for it in range((width + 511) // 512):
                        c0 = lo + it * 512
                        c1 = min(lo + width, c0 + 512)
                        rp = psum_b.tile([64, 512], F32, tag="bank", name="rp")
                        nc.tensor.matmul(
                            rp[:, 0 : c1 - c0],
                            _mm_ap(ones[64:65, 0:64]),
                            _mm_ap(r_sb[64:65, c0:c1]),
                            start=True,
                            stop=True,
                            tile_position=(64, 0),
                        )
                        ce = nc.scalar.copy if it % 2 == 0 else nc.vector.tensor_copy
                        ce(r_sb[0:64, c0:c1], rp[:, 0 : c1 - c0])

                def emit_norm(b_idx):
                    ih, hp = blocks[b_idx]
                    ctx_ps = ctx_ps_of[b_idx]
                    for h2 in (1, 0):
                        r_sb = rpool.tile([65, 1024], MMT, tag="r", name="rsb")
                        nc.vector.reciprocal(
                            out=r_sb[64:65, :], in_=ctx_ps[h2][64:65, :]
                        )
                        bcast_r(r_sb, 0, 1024)
                        if h2 == 0:
                            nc.vector.tensor_tensor(
                                out=ctxn[hp][0:64, ih * 1024 : (ih + 1) * 1024],
                                in0=ctx_ps[0][0:64, :],
                                in1=r_sb[0:64, :],
                                op=mybir.AluOpType.mult,
                            )
                        else:
                            tmp = big.tile([64, 1024], MMT, tag="big", name="tmp")
                            nc.vector.tensor_tensor(
                                out=tmp,
                                in0=ctx_ps[1][0:64, :],
                                in1=r_sb[0:64, :],
                                op=mybir.AluOpType.mult,
                            )
                            # partition shift 0-63 -> 64-127 via SBUF->SBUF DMA
                            nc.sync.dma_start(
                                out=ctxn[hp][64:128, ih * 1024 : (ih + 1) * 1024],
                                in_=tmp,
                            )

                def emit_outproj(ih, mo_list=None, copy_eng=None):
                    for mo in (range(8) if mo_list is None else mo_list):
                        ps = psum_b.tile([128, 1024], F32, tag="bank", name="ops")
                        for nt2 in range(2):
                            nt = 2 * ih + nt2
                            for kt in range(2):
                                nc.tensor.matmul(
                                    ps[:, nt2 * 512 : (nt2 + 1) * 512],
                                    _mm_ap(wo_sb[:, kt, mo * 128 : (mo + 1) * 128]),
                                    _mm_ap(ctxn[kt][:, nt * 512 : (nt + 1) * 512]),
                                    start=(kt == 0),
                                    stop=(kt == 1),
                                )
                        ob = big.tile([128, 1024], MMT, tag="big", name="ob")
                        (copy_eng or nc.vector.tensor_copy)(ob, ps)
                        nc.sync.dma_start(
                            out=outp[
                                mo * 128 : (mo + 1) * 128,
                                ih * 1024 : (ih + 1) * 1024,
                            ],
                            in_=ob,
                        )

                # ---- tail: final block (b3, ih=1/hp=1) normalized and
                # projected in 512-query chunks so the output projection
                # starts as soon as the first chunk's normalize lands.
                def emit_tail_norm(it):
                    ctx_ps = ctx_ps_of[3]
                    cols = slice(it * 512, (it + 1) * 512)
                    dst = slice(1024 + it * 512, 1024 + (it + 1) * 512)
                    for h2 in (1, 0):
                        r_sb = rpool.tile([65, 512], MMT, tag="r", name="rsb")
                        nc.vector.reciprocal(
                            out=r_sb[64:65, :], in_=ctx_ps[h2][64:65, cols]
                        )
                        bcast_r(r_sb, 0, 512)
                        if h2 == 0:
                            nc.vector.tensor_tensor(
                                out=ctxn[1][0:64, dst],
                                in0=ctx_ps[0][0:64, cols],
                                in1=r_sb[0:64, :],
                                op=mybir.AluOpType.mult,
                            )
                        else:
                            tmp = big.tile(
                                [64, 512], MMT, tag="big", name="tmp"
                            )
                            nc.vector.tensor_tensor(
                                out=tmp,
                                in0=ctx_ps[1][0:64, cols],
                                in1=r_sb[0:64, :],
                                op=mybir.AluOpType.mult,
                            )
                            nc.sync.dma_start(
                                out=ctxn[1][64:128, dst], in_=tmp
                            )

                def emit_tail_outproj(it, mo, copy_eng):
                    nt = 2 + it
                    ps = psum_b.tile([128, 512], F32, tag="bank", name="ops")
                    for kt in range(2):
                        nc.tensor.matmul(
                            ps,
                            _mm_ap(wo_sb[:, kt, mo * 128 : (mo + 1) * 128]),
                            _mm_ap(ctxn[kt][:, nt * 512 : (nt + 1) * 512]),
                            start=(kt == 0),
                            stop=(kt == 1),
                        )
                    ob = big.tile([128, 512], MMT, tag="big", name="ob")
                    copy_eng(ob, ps)
                    nc.sync.dma_start(
                        out=outp[
                            mo * 128 : (mo + 1) * 128,
                            nt * 512 : (nt + 1) * 512,
                        ],
                        in_=ob,
                    )

                extras = {}
                for b_idx, (ih, hp) in enumerate(blocks[:3]):
                    last = 16 * (b_idx + 1) - 1
                    extras.setdefault(last + 1, []).append(
                        lambda b=b_idx: emit_norm(b)
                    )
                # ih=0 output projection: spread over the b2 stream and the
                # late (insert-starved) b3 stream
                for j, mo in enumerate(range(8)):
                    pos = 35 + 3 * j if j < 4 else 49 + 3 * (j - 4)
                    extras.setdefault(pos, []).append(
                        lambda m=mo: emit_outproj(0, [m])
                    )

                next_ctx = 0
                for i, u in enumerate(units):
                    emit_s_exp(u)
                    # ctx for a block's first unit lags one extra iteration
                    # so the PE doesn't stall on the previous block's PSUM
                    # (freed only once its normalize has read it)
                    while next_ctx < i and (
                        next_ctx <= i - 2 or units[next_ctx][3] != 0
                    ):
                        emit_ctx(units[next_ctx])
                        next_ctx += 1
                    for fn in inserts.get(i, []):
                        fn()
                    for fn in extras.get(i, []):
                        fn()
                while next_ctx < len(units):
                    emit_ctx(units[next_ctx])
                    next_ctx += 1
                emit_tail_norm(0)
                emit_tail_norm(1)
                for it in range(2):
                    for mo in range(8):
                        ce = nc.scalar.copy if mo % 2 == 0 else nc.vector.tensor_copy
                        emit_tail_outproj(it, mo, ce)

    nc.finalize()
    return nc


_NC_CACHE = None


def _get_program():
    global _NC_CACHE
    if _NC_CACHE is None:
        _NC_CACHE = build_program()
    return _NC_CACHE


def make_in_maps(X, mask, Wq, bq, Wk, bk, Wv, bv, Wo, bo):
    BF16 = ml_dtypes.bfloat16
    X = np.asarray(X, dtype=np.float32)
    mask = np.asarray(mask, dtype=np.float32)
    in_maps = []
    xts = [np.ascontiguousarray(X[b].T).astype(BF16) for b in range(B)]
    mbs = [
        np.ascontiguousarray((-1e6 * (1.0 - mask[b])).reshape(16, 128).T)
        for b in range(B)
    ]
    for c in range(8):
        b, g = c // HG, c % HG
        sl = slice(g * DG, (g + 1) * DG)
        # [p, mt, kt, j] = W[kt*128+p, mt*128+j]
        wq_s = np.ascontiguousarray(
            np.asarray(Wq[:, sl]).reshape(8, 128, 2, 128).transpose(1, 2, 0, 3)
        )
        wk_s = np.ascontiguousarray(
            np.asarray(Wk[:, sl]).reshape(8, 128, 2, 128).transpose(1, 2, 0, 3)
        )
        wv_s = np.ascontiguousarray(
            np.asarray(Wv[:, sl]).reshape(8, 128, DG).transpose(1, 0, 2)
        )
        # Wo rows for this group, pair-packed: [64*h2+p, kt, o] = Wo[g*256+(2kt+h2)*64+p, o]
        wo_s = np.ascontiguousarray(
            np.asarray(Wo[sl, :]).reshape(2, 2, 64, D).transpose(1, 2, 0, 3)
            .reshape(128, 2, D)
        )
        cst_a = np.zeros((128, 20), dtype=np.float32)
        cst_a[:, 0:2] = np.asarray(bq[sl]).reshape(2, 128).T
        cst_a[:, 2:4] = np.asarray(bk[sl]).reshape(2, 128).T
        cst_a[:, 4:20] = mbs[b]
        in_maps.append(
            {
                "xt": xts[b],
                "onesin": np.ones((128, 128), dtype=BF16),
                "bvr": np.asarray(bv[sl]).reshape(1, DG).astype(BF16),
                "cst": cst_a,
                "wq": wq_s.astype(BF16),
                "wk": wk_s.astype(BF16),
                "wv": wv_s.astype(BF16),
                "wo": wo_s.astype(BF16),
            }
        )
    return in_maps


def gather_output(results, bo):
    out = np.zeros((B, N, D), dtype=np.float32)
    for c in range(8):
        out[c // HG] += results[c]["outp"].astype(np.float32).T
    out += np.asarray(bo, dtype=np.float32)
    return out


def kernel(**inputs):
    from concourse import bass_utils

    nc = _get_program()
    in_maps = make_in_maps(**inputs)
    res = bass_utils.run_bass_kernel_spmd(nc, in_maps, core_ids=list(range(8)))
    return gather_output(res.results, inputs["bo"])



# revision 54
# speedup vs baseline: 24.2610x; 2.3606x over previous
"""Multi-head attention (B=2, N=2048, D=1024, H=16, HD=64) on 8 trn2 NeuronCores.

Sharding: data-parallel over batch (2) x tensor-parallel over head groups (4).
Core c handles batch b=c//4, heads 4*(c%4)..4*(c%4)+3. Each core computes
Q/K/V projections for its head slice, attention, and a partial output
projection (its heads' rows of Wo); the host sums the 4 partials per batch
and adds bo.

Device layout strategy: everything lives feature-on-partitions ("transposed")
so no on-device transposes are needed:
  - host passes X[b].T; Q^T/K^T computed as (W^T X^T) with W as stationary.
  - V computed in native [token, d] layout (X^T tiles as stationary).
  - scores computed as S^T[j, i] (key j on partitions) so the mask bias is a
    per-partition scalar and softmax normalization is deferred:
    E^T = exp(S/8 + maskbias) via one ScalarE activation (PSUM->SBUF).
  - ctx^T[d, i] = sum_j V_aug[j, d] E^T[j, i]; V_aug has a ones column so the
    softmax denominator rides along as ctx row 64.
  - normalization multiplies ctx^T by 1/denom broadcast via a tiny PE matmul.
  - out^T = Wo^T ctx^T accumulated over head pairs; host transposes back.

Performance notes (vs the fp32r baseline at 282us):
  - all matmul tensors are bf16 (same 1 cycle/row PE rate, half the DMA
    bytes and SBUF; also far lower sustained power so the part does not
    down-clock under load the way the fp32r version does). PSUM stays fp32;
    bias/mask constants stay fp32. Full-output rel err ~2.3e-3 (gate 2e-2).
  - DMA instruction count is minimized (HWDGE descriptor-gen serializes at
    ~625ns/instruction) and ordered by first use; weights are mt-major so
    each half loads contiguously; small constants are merged into one DMA.
  - the final block's normalize + output projection are 512-query-chunked
    and psum->sbuf copies alternate ScalarE/DVE to shorten the tail drain.
  - ctx accumulation for a block's first unit lags one extra pipeline slot
    so the PE does not stall on the previous block's PSUM release.
"""

import sys

if "/opt/trn_rl_repo" not in sys.path:
    sys.path.insert(0, "/opt/trn_rl_repo")

import ml_dtypes
import numpy as np

import concourse.bacc as bacc
import concourse.mybir as mybir
import concourse.tile as tile

B, N, D = 2, 2048, 1024
H, HD = 16, 64
HG = 4  # head groups (tensor parallel)
HPG = H // HG  # heads per group = 4
DG = HPG * HD  # feature slice per group = 256

F32 = mybir.dt.float32
# Matmul datapath dtype: bf16 runs 1 cycle/row on the PE and halves DMA
# traffic + SBUF footprint vs fp32r. PSUM accumulation stays fp32.
MMT = mybir.dt.bfloat16
# fp32r kept for the small norm-broadcast matmuls (r values stay fp32).
F32R = mybir.dt.float32r


def _mm_ap(ap):
    return ap


def build_program(loop_iters: int = 1):
    nc = bacc.Bacc("TRN2", target_bir_lowering=False)

    xt = nc.dram_tensor("xt", [D, N], MMT, kind="ExternalInput")
    # weights mt-major so each head-pair half loads as one contiguous DMA
    wq = nc.dram_tensor("wq", [128, 2, 8, 128], MMT, kind="ExternalInput")
    wk = nc.dram_tensor("wk", [128, 2, 8, 128], MMT, kind="ExternalInput")
    wv = nc.dram_tensor("wv", [128, 8, DG], MMT, kind="ExternalInput")
    wo = nc.dram_tensor("wo", [128, 2, D], MMT, kind="ExternalInput")
    # merged constants, one DMA: [:,0:2]=bq, [:,2:4]=bk, [:,4:20]=mask bias
    cst = nc.dram_tensor("cst", [128, 20], F32, kind="ExternalInput")
    # bf16 ones: v ones-column source + stationary for PE broadcasts
    onesin = nc.dram_tensor("onesin", [128, 128], MMT, kind="ExternalInput")
    bvr = nc.dram_tensor("bvr", [1, DG], MMT, kind="ExternalInput")
    outp = nc.dram_tensor("outp", [D, N], MMT, kind="ExternalOutput")

    with tile.TileContext(nc) as tc, nc.allow_low_precision(
        reason="fp32r matmul datapath; accumulation stays fp32 in PSUM"
    ):
        import contextlib

        ctx = contextlib.ExitStack()
        with ctx:
            const = ctx.enter_context(tc.tile_pool(name="const", bufs=1))
            big = ctx.enter_context(tc.tile_pool(name="big", bufs=5))
            xtcp = ctx.enter_context(tc.tile_pool(name="xtcp", bufs=4))
            qk = ctx.enter_context(tc.tile_pool(name="qk", bufs=1))
            epool = ctx.enter_context(tc.tile_pool(name="epool", bufs=8))
            rpool = ctx.enter_context(tc.tile_pool(name="rpool", bufs=2))
            psum_b = ctx.enter_context(
                tc.tile_pool(name="psum_b", bufs=2, space="PSUM")
            )
            psum_c = ctx.enter_context(
                tc.tile_pool(name="psum_c", bufs=2, space="PSUM")
            )

            loop_cm = (
                tc.For_i(0, loop_iters, 1)
                if loop_iters > 1
                else contextlib.nullcontext()
            )
            with loop_cm:
                # ---- phase 1 loads, all on the sync queue in first-use
                # order. Each DMA instruction costs ~625ns of serialized
                # HWDGE descriptor generation, so batches are as coarse as
                # first-use order allows.
                wq_sb = big.tile([128, 2, 8, 128], MMT, tag="big")
                wk_sb = big.tile([128, 2, 8, 128], MMT, tag="big")
                wv_sb = big.tile([128, 8, DG], MMT, tag="big")
                xtc_t = [
                    xtcp.tile([128, 8, 512], MMT, tag="xtc", name="xtc")
                    for _ in range(4)
                ]
                xtc = [[xtc_t[c][:, kt, :] for kt in range(8)] for c in range(4)]

                def load_x(c, h0, h1):
                    nc.sync.dma_start(
                        out=xtc_t[c][:, h0:h1, :],
                        in_=xt[
                            h0 * 128 : h1 * 128, c * 512 : (c + 1) * 512
                        ].rearrange("(kt p) col -> p kt col", p=128),
                    )

                # critical prefix: exactly what unit 0's score matmuls need
                nc.sync.dma_start(out=wq_sb[:, 0], in_=wq[:, 0])
                load_x(0, 0, 4)
                load_x(0, 4, 8)
                nc.sync.dma_start(out=wk_sb[:, 0], in_=wk[:, 0])
                load_x(1, 0, 8)

                cst_sb = const.tile([128, 20], F32, tag="cst")
                nc.sync.dma_start(out=cst_sb, in_=cst[:, :])
                ones = const.tile([128, 128], MMT, tag="ones")
                nc.sync.dma_start(out=ones, in_=onesin[:, :])
                bvr_sb = const.tile([1, DG], MMT, tag="bvr")
                nc.sync.dma_start(out=bvr_sb, in_=bvr[:, :])
                bq_sb = cst_sb[:, 0:2]
                bk_sb = cst_sb[:, 2:4]
                mb_sb = cst_sb[:, 4:20]
                wo_sb = const.tile([128, 2, D], MMT, tag="wo")
                bv_bc = const.tile([128, DG], F32, tag="bvbc")

                def emit_bv_bcast():
                    # bv broadcast to all 128 partitions via PE ones-matmul;
                    # deferred into the chain stream so it doesn't head-block
                    # the PE on the cst DMA
                    bv_ps = psum_b.tile([128, DG], F32, tag="bank")
                    nc.tensor.matmul(
                        bv_ps, _mm_ap(ones[0:1, 0:128]), _mm_ap(bvr_sb[0:1, :]),
                        start=True, stop=True,
                    )
                    nc.vector.tensor_copy(bv_bc, bv_ps)

                nc.sync.dma_start(out=wv_sb, in_=wv[:, :, :])

                qt_sb = [qk.tile([128, N], MMT, tag=f"qt{m}", name=f"qt{m}") for m in range(2)]
                kt_sb = [qk.tile([128, N], MMT, tag=f"kt{m}", name=f"kt{m}") for m in range(2)]
                # V with ones column appended per head: [128, jt, head, 65].
                # DVE memset (~0.1us) instead of a strided DMA whose 8k
                # 2-byte descriptors would hold the DMA engines ~3.6us in the
                # critical start window.
                v_sb = qk.tile([128, 16, HPG, HD + 1], MMT, tag="v")
                nc.vector.memset(v_sb[:, :, :, HD : HD + 1], 1.0)
                load_x(2, 0, 8)
                # second head-pair weight columns: first consumer is
                # qk_chain(0/1, 1, *) around unit 11 (~30us in)
                nc.sync.dma_start(out=wq_sb[:, 1], in_=wq[:, 1])
                nc.sync.dma_start(out=wk_sb[:, 1], in_=wk[:, 1])
                load_x(3, 0, 8)
                nc.sync.dma_start(out=wo_sb, in_=wo[:, :, :])

                def qk_chain(proj, mt, nt):
                    w_sb, bias_sb, dst = (
                        (wq_sb, bq_sb, qt_sb) if proj == 0 else (wk_sb, bk_sb, kt_sb)
                    )
                    ps = psum_b.tile([128, 512], F32, tag="bank", name="qkps")
                    for kt in range(8):
                        nc.tensor.matmul(
                            ps,
                            _mm_ap(w_sb[:, mt, kt, :]),
                            _mm_ap(xtc[nt][kt]),
                            start=(kt == 0),
                            stop=(kt == 7),
                        )
                    nc.vector.tensor_scalar_add(
                        dst[mt][:, nt * 512 : (nt + 1) * 512],
                        ps,
                        bias_sb[:, mt : mt + 1],
                    )

                def v_chain(mt):
                    ps = psum_b.tile([128, DG], F32, tag="bank", name="vps")
                    for kt in range(8):
                        nc.tensor.matmul(
                            ps,
                            _mm_ap(
                                xtc[mt // 4][kt][
                                    :, (mt % 4) * 128 : (mt % 4 + 1) * 128
                                ]
                            ),
                            _mm_ap(wv_sb[:, kt, :]),
                            start=(kt == 0),
                            stop=(kt == 7),
                        )
                    nc.vector.tensor_tensor(
                        out=v_sb[:, mt, :, 0:HD],
                        in0=ps.rearrange("p (h d) -> p h d", h=HPG),
                        in1=bv_bc.rearrange("p (h d) -> p h d", h=HPG),
                        op=mybir.AluOpType.add,
                    )

                # chains needed before the unit stream starts (unit 0 only
                # needs Q nt=0/1, K nt=0; V(0) and K nt=1 follow as inserts
                # so the exp stream starts as early as possible)
                for fn in (
                    lambda: qk_chain(0, 0, 0),
                    lambda: qk_chain(1, 0, 0),
                    lambda: qk_chain(0, 0, 1),
                ):
                    fn()

                # remaining chains, spread through the unit stream (key =
                # iteration index at whose END the chain is emitted; each must
                # precede its first consumer unit)
                inserts = {}
                inserts.setdefault(0, []).append(emit_bv_bcast)
                inserts.setdefault(0, []).append(lambda: v_chain(0))
                for i in range(1, 16):
                    inserts.setdefault(i, []).append(lambda m=i: v_chain(m))
                inserts.setdefault(2, []).append(lambda: qk_chain(1, 0, 1))
                inserts.setdefault(6, []).append(lambda: qk_chain(1, 0, 2))
                inserts.setdefault(10, []).append(lambda: qk_chain(1, 0, 3))
                inserts.setdefault(11, []).append(lambda: qk_chain(0, 1, 0))
                inserts.setdefault(12, []).append(lambda: qk_chain(1, 1, 0))
                inserts.setdefault(13, []).append(lambda: qk_chain(0, 1, 1))
                # block boundaries (16/17, 32/33, 48/49) are where the ctx
                # pipeline refills — park chain work there so PE stays fed
                inserts.setdefault(16, []).append(lambda: qk_chain(1, 1, 1))
                inserts.setdefault(17, []).append(lambda: qk_chain(1, 1, 2))
                inserts.setdefault(25, []).append(lambda: qk_chain(1, 1, 3))
                inserts.setdefault(28, []).append(lambda: qk_chain(0, 0, 2))
                inserts.setdefault(30, []).append(lambda: qk_chain(0, 0, 3))
                inserts.setdefault(32, []).append(lambda: qk_chain(0, 1, 2))
                inserts.setdefault(33, []).append(lambda: qk_chain(0, 1, 3))


                # ---- phase 2: attention, software-pipelined emission ----
                ctxn = [
                    qk.tile([128, N], MMT, tag=f"ctxn{m}", name=f"ctxn{m}")
                    for m in range(2)
                ]

                blocks = [(ih, hp) for ih in range(2) for hp in range(2)]
                units = [
                    (b_idx, ih, hp, jt)
                    for b_idx, (ih, hp) in enumerate(blocks)
                    for jt in range(16)
                ]
                ctx_ps_of = {}
                unit_e = {}

                def emit_s_exp(u):
                    b_idx, ih, hp, jt = u
                    e_sb = [
                        epool.tile([128, 1024], MMT, tag="e", name="esb")
                        for _ in range(2)
                    ]
                    s_ps2 = [
                        psum_b.tile([128, 1024], F32, tag="bank", name="sps")
                        for _ in range(2)
                    ]
                    for h2 in range(2):
                        for it in range(2):
                            nc.tensor.matmul(
                                s_ps2[h2][:, it * 512 : (it + 1) * 512],
                                _mm_ap(
                                    kt_sb[hp][
                                        h2 * 64 : (h2 + 1) * 64,
                                        jt * 128 : (jt + 1) * 128,
                                    ]
                                ),
                                _mm_ap(
                                    qt_sb[hp][
                                        h2 * 64 : (h2 + 1) * 64,
                                        ih * 1024 + it * 512 : ih * 1024
                                        + (it + 1) * 512,
                                    ]
                                ),
                                start=True,
                                stop=True,
                            )
                    for h2 in range(2):
                        nc.scalar.activation(
                            out=e_sb[h2],
                            in_=s_ps2[h2],
                            func=mybir.ActivationFunctionType.Exp,
                            bias=mb_sb[:, jt : jt + 1],
                            scale=0.125,
                        )
                    unit_e[u] = e_sb

                def emit_ctx(u):
                    b_idx, ih, hp, jt = u
                    if b_idx not in ctx_ps_of:
                        ctx_ps_of[b_idx] = [
                            psum_c.tile([HD + 1, 1024], F32, tag="ctx", name="ctxps")
                            for _ in range(2)
                        ]
                    ctx_ps = ctx_ps_of[b_idx]
                    e_sb = unit_e.pop(u)
                    for h2 in range(2):
                        for it in range(2):
                            nc.tensor.matmul(
                                ctx_ps[h2][:, it * 512 : (it + 1) * 512],
                                _mm_ap(v_sb[:, jt, 2 * hp + h2, :]),
                                _mm_ap(e_sb[h2][:, it * 512 : (it + 1) * 512]),
                                start=(jt == 0),
                                stop=(jt == 15),
                                skip_group_check=True,
                            )

                def bcast_r(r_sb, lo, width):
                    # broadcast r_sb[64, lo:lo+width] to partitions 0-63 via
                    # a PE ones-matmul; psum->sbuf copies alternate Act/DVE
                    # so the block-boundary latency chain stays short
                    for it in range((width + 511) // 512):
                        c0 = lo + it * 512
                        c1 = min(lo + width, c0 + 512)
                        rp = psum_b.tile([64, 512], F32, tag="bank", name="rp")
                        nc.tensor.matmul(
                            rp[:, 0 : c1 - c0],
                            _mm_ap(ones[64:65, 0:64]),
                            _mm_ap(r_sb[64:65, c0:c1]),
                            start=True,
                            stop=True,
                            tile_position=(64, 0),
                        )
                        ce = nc.scalar.copy if it % 2 == 0 else nc.vector.tensor_copy
                        ce(r_sb[0:64, c0:c1], rp[:, 0 : c1 - c0])

                def emit_norm_h2(b_idx, h2):
                    ih, hp = blocks[b_idx]
                    ctx_ps = ctx_ps_of[b_idx]
                    r_sb = rpool.tile([65, 1024], MMT, tag="r", name="rsb")
                    nc.vector.reciprocal(
                        out=r_sb[64:65, :], in_=ctx_ps[h2][64:65, :]
                    )
                    bcast_r(r_sb, 0, 1024)
                    if h2 == 0:
                        nc.vector.tensor_tensor(
                            out=ctxn[hp][0:64, ih * 1024 : (ih + 1) * 1024],
                            in0=ctx_ps[0][0:64, :],
                            in1=r_sb[0:64, :],
                            op=mybir.AluOpType.mult,
                        )
                    else:
                        tmp = big.tile([64, 1024], MMT, tag="big", name="tmp")
                        nc.vector.tensor_tensor(
                            out=tmp,
                            in0=ctx_ps[1][0:64, :],
                            in1=r_sb[0:64, :],
                            op=mybir.AluOpType.mult,
                        )
                        # partition shift 0-63 -> 64-127 via SBUF->SBUF DMA
                        nc.sync.dma_start(
                            out=ctxn[hp][64:128, ih * 1024 : (ih + 1) * 1024],
                            in_=tmp,
                        )

                def emit_norm(b_idx):
                    emit_norm_h2(b_idx, 1)
                    emit_norm_h2(b_idx, 0)

                def emit_outproj(ih, mo_list=None, copy_eng=None):
                    for mo in (range(8) if mo_list is None else mo_list):
                        ps = psum_b.tile([128, 1024], F32, tag="bank", name="ops")
                        for nt2 in range(2):
                            nt = 2 * ih + nt2
                            for kt in range(2):
                                nc.tensor.matmul(
                                    ps[:, nt2 * 512 : (nt2 + 1) * 512],
                                    _mm_ap(wo_sb[:, kt, mo * 128 : (mo + 1) * 128]),
                                    _mm_ap(ctxn[kt][:, nt * 512 : (nt + 1) * 512]),
                                    start=(kt == 0),
                                    stop=(kt == 1),
                                )
                        ob = big.tile([128, 1024], MMT, tag="big", name="ob")
                        (copy_eng or nc.vector.tensor_copy)(ob, ps)
                        nc.sync.dma_start(
                            out=outp[
                                mo * 128 : (mo + 1) * 128,
                                ih * 1024 : (ih + 1) * 1024,
                            ],
                            in_=ob,
                        )

                # ---- tail: final block (b3, ih=1/hp=1) normalized and
                # projected in 512-query chunks so the output projection
                # starts as soon as the first chunk's normalize lands.
                def emit_tail_norm(it):
                    ctx_ps = ctx_ps_of[3]
                    cols = slice(it * 512, (it + 1) * 512)
                    dst = slice(1024 + it * 512, 1024 + (it + 1) * 512)
                    for h2 in (1, 0):
                        r_sb = rpool.tile([65, 512], MMT, tag="r", name="rsb")
                        nc.vector.reciprocal(
                            out=r_sb[64:65, :], in_=ctx_ps[h2][64:65, cols]
                        )
                        bcast_r(r_sb, 0, 512)
                        if h2 == 0:
                            nc.vector.tensor_tensor(
                                out=ctxn[1][0:64, dst],
                                in0=ctx_ps[0][0:64, cols],
                                in1=r_sb[0:64, :],
                                op=mybir.AluOpType.mult,
                            )
                        else:
                            tmp = big.tile(
                                [64, 512], MMT, tag="big", name="tmp"
                            )
                            nc.vector.tensor_tensor(
                                out=tmp,
                                in0=ctx_ps[1][0:64, cols],
                                in1=r_sb[0:64, :],
                                op=mybir.AluOpType.mult,
                            )
                            nc.sync.dma_start(
                                out=ctxn[1][64:128, dst], in_=tmp
                            )

                def emit_tail_outproj(it, mo, copy_eng):
                    nt = 2 + it
                    ps = psum_b.tile([128, 512], F32, tag="bank", name="ops")
                    for kt in range(2):
                        nc.tensor.matmul(
                            ps,
                            _mm_ap(wo_sb[:, kt, mo * 128 : (mo + 1) * 128]),
                            _mm_ap(ctxn[kt][:, nt * 512 : (nt + 1) * 512]),
                            start=(kt == 0),
                            stop=(kt == 1),
                        )
                    ob = big.tile([128, 512], MMT, tag="big", name="ob")
                    copy_eng(ob, ps)
                    nc.sync.dma_start(
                        out=outp[
                            mo * 128 : (mo + 1) * 128,
                            nt * 512 : (nt + 1) * 512,
                        ],
                        in_=ob,
                    )

                extras = {}
                for b_idx, (ih, hp) in enumerate(blocks[:3]):
                    last = 16 * (b_idx + 1) - 1
                    extras.setdefault(last + 1, []).append(
                        lambda b=b_idx: emit_norm(b)
                    )
                # ih=0 output projection: spread over the b2 stream and the
                # late (insert-starved) b3 stream
                for j, mo in enumerate(range(8)):
                    pos = 35 + 3 * j if j < 4 else 49 + 3 * (j - 4)
                    extras.setdefault(pos, []).append(
                        lambda m=mo: emit_outproj(0, [m])
                    )

                next_ctx = 0
                for i, u in enumerate(units):
                    emit_s_exp(u)
                    # ctx for a block's first unit lags one extra iteration
                    # so the PE doesn't stall on the previous block's PSUM
                    # (freed only once its normalize has read it)
                    while next_ctx < i and (
                        next_ctx <= i - 2 or units[next_ctx][3] != 0
                    ):
                        emit_ctx(units[next_ctx])
                        next_ctx += 1
                    for fn in inserts.get(i, []):
                        fn()
                    for fn in extras.get(i, []):
                        fn()
                while next_ctx < len(units):
                    emit_ctx(units[next_ctx])
                    next_ctx += 1
                emit_tail_norm(0)
                emit_tail_norm(1)
                for it in range(2):
                    for mo in range(8):
                        ce = nc.scalar.copy if mo % 2 == 0 else nc.vector.tensor_copy
                        emit_tail_outproj(it, mo, ce)

    nc.finalize()
    return nc


_NC_CACHE = None


def _get_program():
    global _NC_CACHE
    if _NC_CACHE is None:
        _NC_CACHE = build_program()
    return _NC_CACHE


def make_in_maps(X, mask, Wq, bq, Wk, bk, Wv, bv, Wo, bo):
    BF16 = ml_dtypes.bfloat16
    X = np.asarray(X, dtype=np.float32)
    mask = np.asarray(mask, dtype=np.float32)
    in_maps = []
    xts = [np.ascontiguousarray(X[b].T).astype(BF16) for b in range(B)]
    mbs = [
        np.ascontiguousarray((-1e6 * (1.0 - mask[b])).reshape(16, 128).T)
        for b in range(B)
    ]
    for c in range(8):
        b, g = c // HG, c % HG
        sl = slice(g * DG, (g + 1) * DG)
        # [p, mt, kt, j] = W[kt*128+p, mt*128+j]
        wq_s = np.ascontiguousarray(
            np.asarray(Wq[:, sl]).reshape(8, 128, 2, 128).transpose(1, 2, 0, 3)
        )
        wk_s = np.ascontiguousarray(
            np.asarray(Wk[:, sl]).reshape(8, 128, 2, 128).transpose(1, 2, 0, 3)
        )
        wv_s = np.ascontiguousarray(
            np.asarray(Wv[:, sl]).reshape(8, 128, DG).transpose(1, 0, 2)
        )
        # Wo rows for this group, pair-packed: [64*h2+p, kt, o] = Wo[g*256+(2kt+h2)*64+p, o]
        wo_s = np.ascontiguousarray(
            np.asarray(Wo[sl, :]).reshape(2, 2, 64, D).transpose(1, 2, 0, 3)
            .reshape(128, 2, D)
        )
        cst_a = np.zeros((128, 20), dtype=np.float32)
        cst_a[:, 0:2] = np.asarray(bq[sl]).reshape(2, 128).T
        cst_a[:, 2:4] = np.asarray(bk[sl]).reshape(2, 128).T
        cst_a[:, 4:20] = mbs[b]
        in_maps.append(
            {
                "xt": xts[b],
                "onesin": np.ones((128, 128), dtype=BF16),
                "bvr": np.asarray(bv[sl]).reshape(1, DG).astype(BF16),
                "cst": cst_a,
                "wq": wq_s.astype(BF16),
                "wk": wk_s.astype(BF16),
                "wv": wv_s.astype(BF16),
                "wo": wo_s.astype(BF16),
            }
        )
    return in_maps


def gather_output(results, bo):
    out = np.zeros((B, N, D), dtype=np.float32)
    for c in range(8):
        out[c // HG] += results[c]["outp"].astype(np.float32).T
    out += np.asarray(bo, dtype=np.float32)
    return out


def kernel(**inputs):
    from concourse import bass_utils

    nc = _get_program()
    in_maps = make_in_maps(**inputs)
    res = bass_utils.run_bass_kernel_spmd(nc, in_maps, core_ids=list(range(8)))
    return gather_output(res.results, inputs["bo"])

